# revision 13
# baseline (speedup 1.0000x reference)
"""MetapathAggrNet Trainium2 kernel — 8-core data-parallel over the B axis.

Per core: B_loc=256 examples -> R=3072 paths (r = m*1024 + a*256 + b), each with
8 node tokens / 7 edge tokens. Pipeline (feat-major, fp16 fast path):
  gather node_emb rows (indirect DMA) -> PE-transpose -> X^T = emb^T
  X = scale_W^T @ X^T + scale_b                       (fp16 out)
  phi_k = edge_emb^T @ cum-onehot(etok)  (PE)          cos/sin on ACT
  H_j = prefix sums of rotated x_k                     (DVE fp16)
  P_j = H_j @ W_bot + (j+1)*(x_0 @ W_top)  (PSUM accum; scaled-identity trick)
  a_j = tanh(P_j/(j+1) + attn_b); s_j = ic . a_j  (M=1 matmul)
  softmax over j (deferred exp pass, one ACT table switch)
  mh = relu([x_0; sum_j w_j/(j+1) H_j]); hid_m = sum_a mh
  t_m = sum_b tanh(hid @ inter_W + inter_b)  (ACT accum_out) -> AllReduce
  scores = (t/B) @ inter_context; out = (sum_m hid*scores) @ out_W + out_b
Output written feat-major [2, 128, 256] per core; host reassembles [2048, 256].
"""
import sys

sys.path.insert(0, "/opt/trn_rl_repo")

import numpy as np

import concourse.bass as bass
import concourse.bacc as bacc
import concourse.mybir as mybir
import concourse.tile as tile

P = 128
M, A, B, L = 3, 4, 2048, 7
H = 256
V_N, V_E = 50000, 16
NCORES = 8
BLOC = B // NCORES          # 256
R = M * A * BLOC            # 3072 paths per core
NCH = 6                     # chunks per core
NC = R // NCH               # 512 rows per chunk
NBLK = 32                   # gather blocks per chunk (4096 prows / 128)

f32 = mybir.dt.float32
f32r = mybir.dt.float32r
f16 = mybir.dt.float16
i32 = mybir.dt.int32

_CACHE = {}


def build_nc():
    nc = bacc.Bacc(None, target_bir_lowering=False, debug=False, num_devices=NCORES)

    # ---- I/O ----
    node_emb = nc.dram_tensor("node_emb", [V_N, H], f32, kind="ExternalInput")
    gidx = nc.dram_tensor("gidx", [P, NCH * NBLK], i32, kind="ExternalInput")
    etokT = nc.dram_tensor("etokT", [L, R], i32, kind="ExternalInput")
    scale_W = nc.dram_tensor("scale_W", [H, H], f32, kind="ExternalInput")
    scale_b = nc.dram_tensor("scale_b", [H], f32, kind="ExternalInput")
    attn_W = nc.dram_tensor("attn_W", [2 * H, 2 * H], f32, kind="ExternalInput")
    attn_b = nc.dram_tensor("attn_b", [2 * H], f32, kind="ExternalInput")
    intra_c = nc.dram_tensor("intra_c", [2 * H], f32, kind="ExternalInput")
    inter_W = nc.dram_tensor("inter_W", [2 * H, 2 * H], f32, kind="ExternalInput")
    inter_b = nc.dram_tensor("inter_b", [2 * H], f32, kind="ExternalInput")
    inter_c = nc.dram_tensor("inter_c", [2 * H], f32, kind="ExternalInput")
    out_W = nc.dram_tensor("out_W", [2 * H, H], f32, kind="ExternalInput")
    out_b = nc.dram_tensor("out_b", [H], f32, kind="ExternalInput")
    # host-provided constants
    identc = nc.dram_tensor("identc", [P, P], f32, kind="ExternalInput")
    sel112 = nc.dram_tensor("sel112", [L, 112], f16, kind="ExternalInput")
    iota112 = nc.dram_tensor("iota112", [112, 1], f32, kind="ExternalInput")
    edge_emb = nc.dram_tensor("edge_emb", [V_E, H // 2], f32, kind="ExternalInput")
    ee_rep = nc.dram_tensor("ee_rep", [112, H // 2], f32, kind="ExternalInput")
    jdiv7 = nc.dram_tensor("jdiv7", [L, 1], f32, kind="ExternalInput")
    ones7 = nc.dram_tensor("ones7", [L, 1], f32, kind="ExternalInput")
    OUT = nc.dram_tensor("OUT", [2, P, BLOC], f32, kind="ExternalOutput")

    with tile.TileContext(nc) as tc:
        with (
            tc.tile_pool(name="const", bufs=1) as cp,
            tc.tile_pool(name="persist", bufs=1) as pp,
            tc.tile_pool(name="dram", bufs=1, space="DRAM") as dp,
            tc.tile_pool(name="gp", bufs=4) as gpool,
            tc.tile_pool(name="work", bufs=1) as wp,
            tc.tile_pool(name="hpool", bufs=1) as hp,
            tc.tile_pool(name="cspool", bufs=2) as csp,
            tc.tile_pool(name="p2", bufs=1) as p2,
            tc.tile_pool(name="small", bufs=2) as sp,
            tc.tile_pool(name="cold", bufs=1) as cold,
            tc.tile_pool(name="ps_big", bufs=3, space="PSUM") as ps_big,
            tc.tile_pool(name="ps_tr", bufs=2, space="PSUM") as ps_tr,
            tc.tile_pool(name="ps_s", bufs=1, space="PSUM") as ps_s,
            tc.tile_pool(name="ps_oh", bufs=1, space="PSUM") as ps_oh,
        ):
            # ================= constants / weights prep =================
            ident = cp.tile([P, P], f32)
            nc.sync.dma_start(ident[:], identc[:])
            sel = cp.tile([L, 112], f16)
            nc.sync.dma_start(sel[:], sel112[:])
            iota = cp.tile([112, 1], f32)
            nc.sync.dma_start(iota[:], iota112[:])
            jdiv = cp.tile([L, 1], f32)
            nc.sync.dma_start(jdiv[:], jdiv7[:])

            ee_f = cp.tile([112, H // 2], f32)
            nc.sync.dma_start(ee_f[:], ee_rep[:])
            ee16 = cp.tile([112, P], f16)
            nc.vector.tensor_copy(ee16[:], ee_f[:])

            sw_f = cp.tile([P, 2, H], f32)
            nc.sync.dma_start(sw_f[:], scale_W[:].rearrange("(kb p) m -> p kb m", p=P))
            sw16 = cp.tile([P, 2, H], f16)
            nc.vector.tensor_copy(sw16[:], sw_f[:])
            sb_col = cp.tile([P, 2], f32)
            nc.sync.dma_start(sb_col[:], scale_b[:].rearrange("(mb p) -> p mb", p=P))

            aw_f = cp.tile([P, 4, 2 * H], f32)
            nc.sync.dma_start(aw_f[:], attn_W[:].rearrange("(kb p) m -> p kb m", p=P))
            wt16 = cp.tile([P, 2, 2 * H], f16)
            nc.vector.tensor_copy(wt16[:], aw_f[:, 0:2, :])
            wb16 = cp.tile([P, 2, 2 * H], f16)
            nc.vector.tensor_copy(wb16[:], aw_f[:, 2:4, :])
            ab_col = cp.tile([P, 4], f32)
            nc.sync.dma_start(ab_col[:], attn_b[:].rearrange("(mb p) -> p mb", p=P))
            ic_f = cp.tile([P, 4], f32)
            nc.sync.dma_start(ic_f[:], intra_c[:].rearrange("(mb p) -> p mb", p=P))
            icp = cp.tile([P, 4, P], f16)
            nc.vector.memset(icp[:], 0.0)
            for mb in range(4):
                nc.vector.tensor_copy(icp[:, mb, 0:1], ic_f[:, mb : mb + 1])

            iw_f = cp.tile([P, 4, 2 * H], f32)
            nc.sync.dma_start(iw_f[:], inter_W[:].rearrange("(kb p) m -> p kb m", p=P))
            iw_r = cp.tile([P, 4, 2 * H], f32r)
            nc.vector.tensor_copy(iw_r[:], iw_f[:])
            ib_col = cp.tile([P, 4], f32)
            nc.sync.dma_start(ib_col[:], inter_b[:].rearrange("(mb p) -> p mb", p=P))
            ic2_f = cp.tile([P, 4], f32)
            nc.sync.dma_start(ic2_f[:], inter_c[:].rearrange("(kb p) -> p kb", p=P))
            ic2_r = ic2_f

            ow_f = cp.tile([P, 4, H], f32)
            nc.sync.dma_start(ow_f[:], out_W[:].rearrange("(kb p) m -> p kb m", p=P))
            ow_r = cp.tile([P, 4, H], f32r)
            nc.vector.tensor_copy(ow_r[:], ow_f[:])
            ob_col = cp.tile([P, 2], f32)
            nc.sync.dma_start(ob_col[:], out_b[:].rearrange("(mb p) -> p mb", p=P))

            halfpi = cp.tile([P, 1], f32)
            nc.vector.memset(halfpi[:], float(np.pi / 2))
            zcol = cp.tile([P, 1], f32)
            nc.vector.memset(zcol[:], 0.0)

            ones7_r = cp.tile([L, 1], f32r)
            o7f = cp.tile([L, 1], f32)
            nc.sync.dma_start(o7f[:], ones7[:])
            nc.vector.tensor_copy(ones7_r[:], o7f[:])

            # scaled identities I7[j-1] = I * (j+1), fp16
            I7 = cp.tile([P, L, P], f16)
            for j in range(1, 8):
                nc.vector.tensor_scalar_mul(I7[:, j - 1, :], ident[:], float(j + 1))

            gidx_sb = cp.tile([P, NCH * NBLK], i32)
            nc.sync.dma_start(gidx_sb[:], gidx[:])
            etok_i = cp.tile([L, R], i32)
            nc.sync.dma_start(etok_i[:], etokT[:])
            etok_f = cp.tile([L, R], f32)
            nc.vector.tensor_copy(etok_f[:], etok_i[:])
            etok16 = cp.tile([L, R], f16)
            nc.vector.tensor_copy(etok16[:], etok_f[:])

            # ================= persistent state =================
            hid = pp.tile([P, M, 4, BLOC], f32r)       # per-metapath hidden

            # ================= pass 1: per-chunk =================
            for c in range(NCH):
                # ---- gather + transpose -> XtT (raw emb, feat-major, fp16)
                XtT = wp.tile([P, 2, NBLK * P], f16, tag="XtT")
                for g in range(NBLK):
                    gt = gpool.tile([P, H], f32, tag="g")
                    nc.gpsimd.indirect_dma_start(
                        out=gt[:], out_offset=None, in_=node_emb[:],
                        in_offset=bass.IndirectOffsetOnAxis(
                            ap=gidx_sb[:, c * NBLK + g : c * NBLK + g + 1], axis=0),
                    )
                    for fb in range(2):
                        ptr = ps_tr.tile([P, P], f32, tag="tr")
                        nc.tensor.transpose(
                            ptr[:], gt[:, fb * P : (fb + 1) * P], ident[:])
                        dst = XtT[:, fb, g * P : (g + 1) * P]
                        if g % 2 == 0:
                            nc.scalar.copy(dst, ptr[:])
                        else:
                            nc.vector.tensor_copy(dst, ptr[:])

                # ---- embed matmul: X = scale_W^T @ XtT + scale_b (fp16)
                X = wp.tile([P, 2, NBLK * P], f16, tag="X")
                for q in range(8):
                    cols = slice(q * NC, (q + 1) * NC)
                    for mb in range(2):
                        pe = ps_big.tile([P, NC], f32, tag="big")
                        for kb in range(2):
                            nc.tensor.matmul(
                                pe[:], sw16[:, kb, mb * P : (mb + 1) * P],
                                XtT[:, kb, cols], start=(kb == 0), stop=(kb == 1))
                        nc.scalar.activation(
                            X[:, mb, cols], pe[:],
                            mybir.ActivationFunctionType.Identity,
                            bias=sb_col[:, mb : mb + 1], scale=1.0)
                # save x_0 (k=0 block is cols 0:512)
                x0c = hp.tile([P, 2, NC], f16, tag="x0")
                nc.vector.tensor_copy(x0c[:], X[:, :, 0:NC])
                Hs = hp.tile([P, 2, L, NC], f16, tag="Hs")

                # ---- phi + cos/sin
                oh = cold.tile([112, NC], f16, tag="oh")
                poh = ps_oh.tile([112, NC], f32, tag="oh")
                nc.tensor.matmul(
                    poh[:], sel[:], etok16[:, c * NC : (c + 1) * NC],
                    start=True, stop=True)
                nc.vector.tensor_tensor(
                    out=oh[:], in0=poh[:], in1=iota[:].to_broadcast([112, NC]),
                    op=mybir.AluOpType.is_equal)
                # ---- phi -> cos/sin -> rotation + prefix sums H_j
                for k in range(1, 8):
                    pphi = ps_big.tile([P, NC], f32, tag="big")
                    nc.tensor.matmul(
                        pphi[:], ee16[: 16 * k, :], oh[: 16 * k, :],
                        start=True, stop=True)
                    csk = csp.tile([P, 2, NC], f16, tag="csk")
                    nc.scalar.activation(
                        csk[:, 0, :], pphi[:],
                        mybir.ActivationFunctionType.Sin,
                        bias=halfpi[:, 0:1], scale=1.0)
                    nc.scalar.activation(
                        csk[:, 1, :], pphi[:],
                        mybir.ActivationFunctionType.Sin, bias=zcol[:, 0:1], scale=1.0)
                    xr = X[:, 0, k * NC : (k + 1) * NC]
                    xi = X[:, 1, k * NC : (k + 1) * NC]
                    ck = csk[:, 0, :]
                    sk = csk[:, 1, :]
                    t1 = sp.tile([P, NC], f16, tag="t1")
                    t2 = sp.tile([P, NC], f16, tag="t2")
                    hprev_r = x0c[:, 0, :] if k == 1 else Hs[:, 0, k - 2, :]
                    hprev_i = x0c[:, 1, :] if k == 1 else Hs[:, 1, k - 2, :]
                    # H_r = Hprev_r + xr*c - xi*s
                    nc.vector.tensor_tensor(out=t1[:], in0=xr, in1=ck,
                                            op=mybir.AluOpType.mult)
                    nc.vector.tensor_tensor(out=t2[:], in0=xi, in1=sk,
                                            op=mybir.AluOpType.mult)
                    nc.vector.tensor_tensor(out=t1[:], in0=t1[:], in1=hprev_r,
                                            op=mybir.AluOpType.add)
                    nc.vector.tensor_tensor(out=Hs[:, 0, k - 1, :], in0=t1[:],
                                            in1=t2[:], op=mybir.AluOpType.subtract)
                    # H_i = Hprev_i + xr*s + xi*c
                    nc.vector.tensor_tensor(out=t1[:], in0=xr, in1=sk,
                                            op=mybir.AluOpType.mult)
                    nc.vector.tensor_tensor(out=t2[:], in0=xi, in1=ck,
                                            op=mybir.AluOpType.mult)
                    nc.vector.tensor_tensor(out=t1[:], in0=t1[:], in1=hprev_i,
                                            op=mybir.AluOpType.add)
                    nc.vector.tensor_tensor(out=Hs[:, 1, k - 1, :], in0=t1[:],
                                            in1=t2[:], op=mybir.AluOpType.add)

                # ---- u = x_0 @ W_top (fp16)
                u16 = cold.tile([P, 4, NC], f16, tag="u16")
                for mb in range(4):
                    pu = ps_big.tile([P, NC], f32, tag="big")
                    for kb in range(2):
                        nc.tensor.matmul(
                            pu[:], wt16[:, kb, mb * P : (mb + 1) * P],
                            x0c[:, kb, :], start=(kb == 0), stop=(kb == 1))
                    nc.scalar.copy(u16[:, mb, :], pu[:])

                # ---- attention P_j + tanh + ic-dot + per-j exp
                e_all = cold.tile([1, L, NC], f32, tag="e_all")
                for j in range(1, 8):
                    psj = ps_s.tile([P, NC], f32, tag="s")
                    a4 = cold.tile([P, 4, NC], f16, tag="a4")
                    for mb in range(4):
                        pP = ps_big.tile([P, NC], f32, tag="big")
                        for kb in range(2):
                            nc.tensor.matmul(
                                pP[:], wb16[:, kb, mb * P : (mb + 1) * P],
                                Hs[:, kb, j - 1, :],
                                start=(kb == 0), stop=False)
                        nc.tensor.matmul(
                            pP[:], I7[:, j - 1, :], u16[:, mb, :],
                            start=False, stop=True)
                        nc.scalar.activation(
                            a4[:, mb, :], pP[:], mybir.ActivationFunctionType.Tanh,
                            bias=ab_col[:, mb : mb + 1], scale=float(1.0 / (j + 1)))
                    for mb in range(4):
                        nc.tensor.matmul(
                            psj[:], icp[:, mb, :], a4[:, mb, :],
                            start=(mb == 0), stop=(mb == 3))
                    nc.scalar.activation(e_all[:, j - 1, :], psj[0:1, :],
                                         mybir.ActivationFunctionType.Exp,
                                         bias=zcol[0:1, 0:1], scale=1.0)

                # ---- softmax over j (per-row) + weighted H sum + hid accum
                esum = cold.tile([1, NC], f32, tag="esum")
                nc.vector.tensor_tensor(out=esum[:], in0=e_all[:, 0, :],
                                        in1=e_all[:, 1, :], op=mybir.AluOpType.add)
                for j in range(3, 8):
                    nc.vector.tensor_tensor(out=esum[:], in0=esum[:],
                                            in1=e_all[:, j - 1, :],
                                            op=mybir.AluOpType.add)
                rec = cold.tile([1, NC], f32, tag="rec")
                nc.vector.reciprocal(rec[:], esum[:])
                wdd_all = cold.tile([1, L, NC], f16, tag="wdd_all")
                for j in range(1, 8):
                    wj = cold.tile([1, NC], f32, tag="wj")
                    nc.vector.tensor_tensor(out=wj[:], in0=e_all[:, j - 1, :],
                                            in1=rec[:], op=mybir.AluOpType.mult)
                    nc.vector.tensor_scalar_mul(wdd_all[:, j - 1, :], wj[:],
                                                float(1.0 / (j + 1)))

                mh = cold.tile([P, 2, NC], f16, tag="mh")
                for j in range(1, 8):
                    wbc = sp.tile([P, NC], f16, tag="wbc")
                    nc.gpsimd.partition_broadcast(wbc[:], wdd_all[:, j - 1, :])
                    for fb in range(2):
                        if j == 1:
                            nc.vector.tensor_tensor(
                                out=mh[:, fb, :], in0=Hs[:, fb, 0, :],
                                in1=wbc[:], op=mybir.AluOpType.mult)
                        else:
                            tmp = sp.tile([P, NC], f16, tag="tmp")
                            nc.vector.tensor_tensor(
                                out=tmp[:], in0=Hs[:, fb, j - 1, :],
                                in1=wbc[:], op=mybir.AluOpType.mult)
                            nc.vector.tensor_tensor(
                                out=mh[:, fb, :], in0=mh[:, fb, :], in1=tmp[:],
                                op=mybir.AluOpType.add)
                m = c // 2
                for ah in range(2):
                    cols = slice(ah * BLOC, (ah + 1) * BLOC)
                    first = (c % 2 == 0) and (ah == 0)
                    for g2 in range(2):  # 0: top (x0), 1: bottom (mh)
                        srct = x0c[:, :, cols] if g2 == 0 else mh[:, :, cols]
                        rel = cold.tile([P, 2, BLOC], f32r, tag="rel")
                        nc.vector.tensor_scalar(
                            out=rel[:], in0=srct, scalar1=0.0, scalar2=None,
                            op0=mybir.AluOpType.max)
                        dst = hid[:, m, 2 * g2 : 2 * g2 + 2, :]
                        if first:
                            nc.vector.tensor_copy(dst, rel[:])
                        else:
                            nc.vector.tensor_tensor(
                                out=dst, in0=dst, in1=rel[:],
                                op=mybir.AluOpType.add)

            # ---- inter attention: t_m = sum_b tanh(hid_m @ inter_W + ib)
            tpart = pp.tile([P, 4, M], f32)
            scratch = p2.tile([P, BLOC], f32, tag="scr")
            for m in range(M):
                for mb in range(4):
                    pt = ps_big.tile([P, BLOC], f32, tag="big")
                    for kb in range(4):
                        nc.tensor.matmul(
                            pt[:], iw_r[:, kb, mb * P : (mb + 1) * P],
                            hid[:, m, kb, :], start=(kb == 0), stop=(kb == 3))
                    nc.scalar.activation(
                        scratch[:], pt[:], mybir.ActivationFunctionType.Tanh,
                        bias=ib_col[:, mb : mb + 1], scale=1.0,
                        accum_out=tpart[:, mb, m : m + 1])

            # ---- AllReduce t over cores
            t_in = dp.tile([P, 4 * M], f32)
            t_out = dp.tile([P, 4 * M], f32, addr_space="Shared")
            nc.sync.dma_start(t_in[:], tpart[:].rearrange("p a b -> p (a b)"))
            nc.gpsimd.collective_compute(
                "AllReduce", mybir.AluOpType.add,
                replica_groups=[list(range(NCORES))],
                ins=[t_in[:]], outs=[t_out[:]])
            tsum = p2.tile([P, 4, M], f32, tag="tsum")
            nc.sync.dma_start(tsum[:], t_out[:].rearrange("p (a b) -> p a b", a=4))
            tsum_r = p2.tile([P, 4, M], f32, tag="tsr")
            nc.vector.tensor_scalar_mul(tsum_r[:], tsum[:], float(1.0 / B))

            # ---- scores
            psc = ps_s.tile([1, M], f32, tag="sc")
            for kb in range(4):
                nc.tensor.matmul(psc[:], ic2_r[:, kb : kb + 1], tsum_r[:, kb, :],
                                 start=(kb == 0), stop=(kb == 3))
            sc_sb = p2.tile([1, M], f32, tag="scsb")
            nc.vector.tensor_copy(sc_sb[:], psc[:])
            scb = p2.tile([P, M], f32, tag="scb")
            nc.gpsimd.partition_broadcast(scb[:], sc_sb[:])

            # ---- out_pre = sum_m hid_m * scores_m ; head matmul
            op_r = p2.tile([P, 4, BLOC], f32r, tag="opr")
            tmp2 = p2.tile([P, 4, BLOC], f32r, tag="tmp2")
            for m in range(M):
                dst = op_r if m == 0 else tmp2
                nc.vector.tensor_scalar(
                    out=dst[:], in0=hid[:, m, :, :], scalar1=scb[:, m : m + 1],
                    scalar2=None, op0=mybir.AluOpType.mult)
                if m > 0:
                    nc.vector.tensor_tensor(out=op_r[:], in0=op_r[:], in1=tmp2[:],
                                            op=mybir.AluOpType.add)
            outsb = p2.tile([P, 2, BLOC], f32, tag="outsb")
            for mb in range(2):
                po = ps_big.tile([P, BLOC], f32, tag="big")
                for kb in range(4):
                    nc.tensor.matmul(
                        po[:], ow_r[:, kb, mb * P : (mb + 1) * P],
                        op_r[:, kb, :], start=(kb == 0), stop=(kb == 3))
                nc.scalar.activation(
                    outsb[:, mb, :], po[:], mybir.ActivationFunctionType.Identity,
                    bias=ob_col[:, mb : mb + 1], scale=1.0)
            nc.sync.dma_start(OUT[:].rearrange("b p n -> p b n"), outsb[:])

    nc.compile()
    return nc


# ---------------- host side ----------------

def _host_prep(inputs):
    tokens = np.asarray(inputs["tokens"]).astype(np.int32)       # [3,4,2048,8]
    etok = np.asarray(inputs["edge_tokens"]).astype(np.int32)    # [3,4,2048,7]
    consts = {
        "identc": np.eye(P, dtype=np.float32),
        "sel112": np.repeat(np.eye(L, dtype=np.float16), 16, axis=1),
        "iota112": np.tile(np.arange(16, dtype=np.float32), L)[:, None],
        "jdiv7": (1.0 / np.arange(2, 9, dtype=np.float32))[:, None],
        "ones7": np.ones((L, 1), np.float32),
    }
    rep = {k: np.ascontiguousarray(np.asarray(inputs[k]).astype(np.float32))
           for k in ["node_emb", "edge_emb", "scale_W", "scale_b", "attn_W",
                      "attn_b", "inter_W", "inter_b", "out_W", "out_b"]}
    rep["intra_c"] = np.asarray(inputs["intra_context"]).astype(np.float32)
    rep["ee_rep"] = np.ascontiguousarray(
        np.tile(np.asarray(inputs["edge_emb"]).astype(np.float32), (L, 1)))
    rep["inter_c"] = np.asarray(inputs["inter_context"]).astype(np.float32)

    in_maps = []
    for core in range(NCORES):
        bs = slice(core * BLOC, (core + 1) * BLOC)
        tok_r = tokens[:, :, bs, :].reshape(R, L + 1)            # r=(m,a,b)
        et_r = etok[:, :, bs, :].reshape(R, L)
        # gidx[p, c*32 + k*4 + q] = tok_r[c*512 + q*128 + p, k]
        t4 = tok_r.reshape(NCH, 4, P, L + 1)                     # [c, q, p, k]
        gidx = np.ascontiguousarray(
            t4.transpose(2, 0, 3, 1).reshape(P, NCH * (L + 1) * 4))
        etokT = np.ascontiguousarray(et_r.T)                     # [7, R]
        m = {"gidx": gidx, "etokT": etokT}
        m.update(consts)
        m.update(rep)
        in_maps.append(m)
    return in_maps


def _get_runner():
    if "runner" not in _CACHE:
        nc = build_nc()
        from concourse import bass2jax  # noqa
        import jax
        import jax.numpy as jnp
        from jax.experimental.shard_map import shard_map
        from jax.sharding import Mesh, PartitionSpec
        from concourse.bass2jax import (_bass_exec_p, install_neuronx_cc_hook,
                                        partition_id_tensor)
        install_neuronx_cc_hook()
        partition_name = (nc.partition_id_tensor.name
                          if nc.partition_id_tensor else None)
        in_names, out_names, out_avals = [], [], []
        for alloc in nc.m.functions[0].allocations:
            if not isinstance(alloc, mybir.MemoryLocationSet):
                continue
            name = alloc.memorylocations[0].name
            if alloc.kind == "ExternalInput":
                if name != partition_name:
                    in_names.append(name)
            elif alloc.kind == "ExternalOutput":
                shape = tuple(alloc.tensor_shape)
                npdt = mybir.dt.np(alloc.dtype)
                out_names.append(name)
                out_avals.append(jax.core.ShapedArray(shape, npdt))
        n_params, n_outs = len(in_names), len(out_avals)
        all_in = in_names + out_names + ([partition_name] if partition_name else [])

        def _body(*args):
            operands = list(args)
            if partition_name is not None:
                operands.append(partition_id_tensor())
            return tuple(_bass_exec_p.bind(
                *operands, out_avals=tuple(out_avals), in_names=tuple(all_in),
                out_names=tuple(out_names), lowering_input_output_aliases=(),
                sim_require_finite=True, sim_require_nnan=True, nc=nc))

        devices = jax.devices()[:NCORES]
        mesh = Mesh(np.asarray(devices), ("core",))
        fn = jax.jit(
            shard_map(_body, mesh=mesh,
                      in_specs=(PartitionSpec("core"),) * (n_params + n_outs),
                      out_specs=(PartitionSpec("core"),) * n_outs,
                      check_rep=False),
            donate_argnums=tuple(range(n_params, n_params + n_outs)),
            keep_unused=True)
        _CACHE["runner"] = (fn, in_names, out_names, out_avals)
    return _CACHE["runner"]


def run_device(in_maps):
    fn, in_names, out_names, out_avals = _get_runner()
    concat = [np.concatenate([np.asarray(in_maps[c][n]) for c in range(NCORES)],
                             axis=0) for n in in_names]
    zeros = [np.zeros((NCORES * a.shape[0], *a.shape[1:]), a.dtype)
             for a in out_avals]
    outs = fn(*concat, *zeros)
    outs = [np.asarray(o) for o in outs]
    return [
        {name: outs[i].reshape(NCORES, *out_avals[i].shape)[c]
         for i, name in enumerate(out_names)}
        for c in range(NCORES)
    ]


def kernel(**inputs):
    in_maps = _host_prep(inputs)
    res = run_device(in_maps)
    full = np.empty((B, H), np.float32)
    for core in range(NCORES):
        o = res[core]["OUT"]                      # [2, 128, 256] = [mb, p, b]
        full[core * BLOC : (core + 1) * BLOC, :] = (
            o.transpose(2, 0, 1).reshape(BLOC, H))
    return full



# revision 34
# speedup vs baseline: 8.1298x; 8.1298x over previous
"""MetapathAggrNet Trainium2 kernel — 8-core data-parallel over the B axis.

Per core: B_loc=256 examples -> R=3072 paths (r = m*1024 + a*256 + b), each with
8 node tokens / 7 edge tokens. Pipeline (feat-major, fp16 fast path):
  gather node_emb rows (indirect DMA) -> PE-transpose -> X^T = emb^T
  X = scale_W^T @ X^T + scale_b                       (fp16 out)
  phi_k = edge_emb^T @ cum-onehot(etok)  (PE)          cos/sin on ACT
  H_j = prefix sums of rotated x_k                     (DVE fp16)
  P_j = H_j @ W_bot + (j+1)*(x_0 @ W_top)  (PSUM accum; scaled-identity trick)
  a_j = tanh(P_j/(j+1) + attn_b); s_j = ic . a_j  (M=1 matmul)
  softmax over j (deferred exp pass, one ACT table switch)
  mh = relu([x_0; sum_j w_j/(j+1) H_j]); hid_m = sum_a mh
  t_m = sum_b tanh(hid @ inter_W + inter_b)  (ACT accum_out) -> AllReduce
  scores = (t/B) @ inter_context; out = (sum_m hid*scores) @ out_W + out_b
Output written feat-major [2, 128, 256] per core; host reassembles [2048, 256].
"""
import sys

sys.path.insert(0, "/opt/trn_rl_repo")

import numpy as np

import concourse.bass as bass
import concourse.bacc as bacc
import concourse.mybir as mybir
import concourse.tile as tile

P = 128
M, A, B, L = 3, 4, 2048, 7
H = 256
V_N, V_E = 50000, 16
NCORES = 8
BLOC = B // NCORES          # 256
R = M * A * BLOC            # 3072 paths per core
NCH = 6                     # chunks per core
NC = R // NCH               # 512 rows per chunk
NBLK = 32                   # gather blocks per chunk (4096 prows / 128)

f32 = mybir.dt.float32
f32r = mybir.dt.float32r
f16 = mybir.dt.float16
i32 = mybir.dt.int32

_CACHE = {}


def build_nc():
    nc = bacc.Bacc(None, target_bir_lowering=False, debug=False, num_devices=NCORES)

    # ---- I/O ----
    node_emb = nc.dram_tensor("node_emb", [V_N, H], f16, kind="ExternalInput")
    gidx = nc.dram_tensor("gidx", [P, NCH * NBLK], i32, kind="ExternalInput")
    etokT = nc.dram_tensor("etokT", [L, R], i32, kind="ExternalInput")
    scale_W = nc.dram_tensor("scale_W", [H, H], f32, kind="ExternalInput")
    scale_b = nc.dram_tensor("scale_b", [H], f32, kind="ExternalInput")
    attn_W = nc.dram_tensor("attn_W", [2 * H, 2 * H], f32, kind="ExternalInput")
    attn_b = nc.dram_tensor("attn_b", [2 * H], f32, kind="ExternalInput")
    intra_c = nc.dram_tensor("intra_c", [2 * H], f32, kind="ExternalInput")
    inter_W = nc.dram_tensor("inter_W", [2 * H, 2 * H], f32, kind="ExternalInput")
    inter_b = nc.dram_tensor("inter_b", [2 * H], f32, kind="ExternalInput")
    inter_c = nc.dram_tensor("inter_c", [2 * H], f32, kind="ExternalInput")
    out_W = nc.dram_tensor("out_W", [2 * H, H], f32, kind="ExternalInput")
    out_b = nc.dram_tensor("out_b", [H], f32, kind="ExternalInput")
    # host-provided constants
    identc = nc.dram_tensor("identc", [P, P], f32, kind="ExternalInput")
    sel112 = nc.dram_tensor("sel112", [L, 112], f16, kind="ExternalInput")
    iota112 = nc.dram_tensor("iota112", [112, 1], f32, kind="ExternalInput")
    edge_emb = nc.dram_tensor("edge_emb", [V_E, H // 2], f32, kind="ExternalInput")
    ee_rep = nc.dram_tensor("ee_rep", [112, H // 2], f32, kind="ExternalInput")
    jdiv7 = nc.dram_tensor("jdiv7", [L, 1], f32, kind="ExternalInput")
    ones7 = nc.dram_tensor("ones7", [L, 1], f32, kind="ExternalInput")
    OUT = nc.dram_tensor("OUT", [2, P, BLOC], f32, kind="ExternalOutput")

    with tile.TileContext(nc) as tc:
        with (
            tc.tile_pool(name="const", bufs=1) as cp,
            tc.tile_pool(name="persist", bufs=1) as pp,
            tc.tile_pool(name="dram", bufs=1, space="DRAM") as dp,
            tc.tile_pool(name="gp", bufs=4) as gpool,
            tc.tile_pool(name="work", bufs=1) as wp,
            tc.tile_pool(name="hpool", bufs=1) as hp,
            tc.tile_pool(name="cspool", bufs=2) as csp,
            tc.tile_pool(name="p2", bufs=1) as p2,
            tc.tile_pool(name="small", bufs=2) as sp,
            tc.tile_pool(name="cold", bufs=1) as cold,
            tc.tile_pool(name="ps_big", bufs=3, space="PSUM") as ps_big,
            tc.tile_pool(name="ps_tr", bufs=2, space="PSUM") as ps_tr,
            tc.tile_pool(name="ps_s", bufs=1, space="PSUM") as ps_s,
            tc.tile_pool(name="ps_oh", bufs=1, space="PSUM") as ps_oh,
        ):
            # ================= constants / weights prep =================
            ident = cp.tile([P, P], f32)
            nc.sync.dma_start(ident[:], identc[:])
            sel = cp.tile([L, 112], f16)
            nc.sync.dma_start(sel[:], sel112[:])
            iota = cp.tile([112, 1], f32)
            nc.sync.dma_start(iota[:], iota112[:])
            jdiv = cp.tile([L, 1], f32)
            nc.sync.dma_start(jdiv[:], jdiv7[:])

            ee_f = cp.tile([112, H // 2], f32)
            nc.sync.dma_start(ee_f[:], ee_rep[:])
            ee16 = cp.tile([112, P], f16)
            nc.vector.tensor_copy(ee16[:], ee_f[:])

            sw_f = cp.tile([P, 2, H], f32)
            nc.sync.dma_start(sw_f[:], scale_W[:].rearrange("(kb p) m -> p kb m", p=P))
            sw16 = cp.tile([P, 2, H], f16)
            nc.vector.tensor_copy(sw16[:], sw_f[:])
            sb_col = cp.tile([P, 2], f32)
            nc.sync.dma_start(sb_col[:], scale_b[:].rearrange("(mb p) -> p mb", p=P))

            aw_f = cp.tile([P, 4, 2 * H], f32)
            nc.sync.dma_start(aw_f[:], attn_W[:].rearrange("(kb p) m -> p kb m", p=P))
            wt16 = cp.tile([P, 2, 2 * H], f16)
            nc.vector.tensor_copy(wt16[:], aw_f[:, 0:2, :])
            wb16 = cp.tile([P, 2, 2 * H], f16)
            nc.vector.tensor_copy(wb16[:], aw_f[:, 2:4, :])
            ab_col = cp.tile([P, 4], f32)
            nc.sync.dma_start(ab_col[:], attn_b[:].rearrange("(mb p) -> p mb", p=P))
            ic_f = cp.tile([P, 4], f32)
            nc.sync.dma_start(ic_f[:], intra_c[:].rearrange("(mb p) -> p mb", p=P))
            icp = cp.tile([P, 4, P], f16)
            nc.vector.memset(icp[:], 0.0)
            for mb in range(4):
                nc.vector.tensor_copy(icp[:, mb, 0:1], ic_f[:, mb : mb + 1])

            iw_f = cp.tile([P, 4, 2 * H], f32)
            nc.sync.dma_start(iw_f[:], inter_W[:].rearrange("(kb p) m -> p kb m", p=P))
            iw_r = cp.tile([P, 4, 2 * H], f32r)
            nc.vector.tensor_copy(iw_r[:], iw_f[:])
            ib_col = cp.tile([P, 4], f32)
            nc.sync.dma_start(ib_col[:], inter_b[:].rearrange("(mb p) -> p mb", p=P))
            ic2_f = cp.tile([P, 4], f32)
            nc.sync.dma_start(ic2_f[:], inter_c[:].rearrange("(kb p) -> p kb", p=P))
            ic2_r = ic2_f

            ow_f = cp.tile([P, 4, H], f32)
            nc.sync.dma_start(ow_f[:], out_W[:].rearrange("(kb p) m -> p kb m", p=P))
            ow_r = cp.tile([P, 4, H], f32r)
            nc.vector.tensor_copy(ow_r[:], ow_f[:])
            ob_col = cp.tile([P, 2], f32)
            nc.sync.dma_start(ob_col[:], out_b[:].rearrange("(mb p) -> p mb", p=P))

            halfpi = cp.tile([P, 1], f32)
            nc.vector.memset(halfpi[:], float(np.pi / 2))
            zcol = cp.tile([P, 1], f32)
            nc.vector.memset(zcol[:], 0.0)

            ones7_r = cp.tile([L, 1], f32r)
            o7f = cp.tile([L, 1], f32)
            nc.sync.dma_start(o7f[:], ones7[:])
            nc.vector.tensor_copy(ones7_r[:], o7f[:])

            # scaled identities I7[j-1] = I * (j+1), fp16
            I7 = cp.tile([P, L, P], f16)
            for j in range(1, 8):
                nc.vector.tensor_scalar_mul(I7[:, j - 1, :], ident[:], float(j + 1))

            gidx_sb = cp.tile([P, NCH * NBLK], i32)
            nc.sync.dma_start(gidx_sb[:], gidx[:])
            etok_i = cp.tile([L, R], i32)
            nc.sync.dma_start(etok_i[:], etokT[:])
            etok_f = cp.tile([L, R], f32)
            nc.vector.tensor_copy(etok_f[:], etok_i[:])
            etok16 = cp.tile([L, R], f16)
            nc.vector.tensor_copy(etok16[:], etok_f[:])

            # ================= persistent state =================
            hid = pp.tile([P, M, 4, BLOC], f32r)       # per-metapath hidden

            # ================= pass 1: per-chunk =================
            for c in range(NCH):
                # ---- gather + transpose -> XtT (raw emb, feat-major, fp16)
                XtT = wp.tile([P, 2, NBLK * P], f16, tag="XtT")
                for g in range(NBLK):
                    gt16 = gpool.tile([P, H], f16, tag="g16")
                    nc.gpsimd.indirect_dma_start(
                        out=gt16[:], out_offset=None, in_=node_emb[:],
                        in_offset=bass.IndirectOffsetOnAxis(
                            ap=gidx_sb[:, c * NBLK + g : c * NBLK + g + 1], axis=0),
                    )
                    gt = gpool.tile([P, H], f32, tag="g")
                    nc.vector.tensor_copy(gt[:], gt16[:])
                    for fb in range(2):
                        ptr = ps_tr.tile([P, P], f32, tag="tr")
                        nc.tensor.transpose(
                            ptr[:], gt[:, fb * P : (fb + 1) * P], ident[:])
                        dst = XtT[:, fb, g * P : (g + 1) * P]
                        if g % 2 == 0:
                            nc.scalar.copy(dst, ptr[:])
                        else:
                            nc.vector.tensor_copy(dst, ptr[:])

                # ---- embed matmul: X = scale_W^T @ XtT + scale_b (fp16)
                X = wp.tile([P, 2, NBLK * P], f16, tag="X")
                for q in range(8):
                    cols = slice(q * NC, (q + 1) * NC)
                    for mb in range(2):
                        pe = ps_big.tile([P, NC], f32, tag="big")
                        for kb in range(2):
                            nc.tensor.matmul(
                                pe[:], sw16[:, kb, mb * P : (mb + 1) * P],
                                XtT[:, kb, cols], start=(kb == 0), stop=(kb == 1))
                        nc.scalar.activation(
                            X[:, mb, cols], pe[:],
                            mybir.ActivationFunctionType.Identity,
                            bias=sb_col[:, mb : mb + 1], scale=1.0)
                # save x_0 (k=0 block is cols 0:512)
                x0c = hp.tile([P, 2, NC], f16, tag="x0")
                nc.vector.tensor_copy(x0c[:], X[:, :, 0:NC])
                Hs = hp.tile([P, 2, L, NC], f16, tag="Hs")

                # ---- phi + cos/sin
                oh = cold.tile([112, NC], f16, tag="oh")
                poh = ps_oh.tile([112, NC], f32, tag="oh")
                nc.tensor.matmul(
                    poh[:], sel[:], etok16[:, c * NC : (c + 1) * NC],
                    start=True, stop=True)
                nc.vector.tensor_tensor(
                    out=oh[:], in0=poh[:], in1=iota[:].to_broadcast([112, NC]),
                    op=mybir.AluOpType.is_equal)
                # ---- phi -> cos/sin -> rotation + prefix sums H_j
                for k in range(1, 8):
                    pphi = ps_big.tile([P, NC], f32, tag="big")
                    nc.tensor.matmul(
                        pphi[:], ee16[: 16 * k, :], oh[: 16 * k, :],
                        start=True, stop=True)
                    csk = csp.tile([P, 2, NC], f16, tag="csk")
                    nc.scalar.activation(
                        csk[:, 0, :], pphi[:],
                        mybir.ActivationFunctionType.Sin,
                        bias=halfpi[:, 0:1], scale=1.0)
                    nc.scalar.activation(
                        csk[:, 1, :], pphi[:],
                        mybir.ActivationFunctionType.Sin, bias=zcol[:, 0:1], scale=1.0)
                    xr = X[:, 0, k * NC : (k + 1) * NC]
                    xi = X[:, 1, k * NC : (k + 1) * NC]
                    ck = csk[:, 0, :]
                    sk = csk[:, 1, :]
                    t1 = sp.tile([P, NC], f16, tag="t1")
                    t2 = sp.tile([P, NC], f16, tag="t2")
                    hprev_r = x0c[:, 0, :] if k == 1 else Hs[:, 0, k - 2, :]
                    hprev_i = x0c[:, 1, :] if k == 1 else Hs[:, 1, k - 2, :]
                    # H_r = Hprev_r + xr*c - xi*s
                    nc.vector.tensor_tensor(out=t1[:], in0=xr, in1=ck,
                                            op=mybir.AluOpType.mult)
                    nc.vector.tensor_tensor(out=t2[:], in0=xi, in1=sk,
                                            op=mybir.AluOpType.mult)
                    nc.vector.tensor_tensor(out=t1[:], in0=t1[:], in1=hprev_r,
                                            op=mybir.AluOpType.add)
                    nc.vector.tensor_tensor(out=Hs[:, 0, k - 1, :], in0=t1[:],
                                            in1=t2[:], op=mybir.AluOpType.subtract)
                    # H_i = Hprev_i + xr*s + xi*c
                    nc.vector.tensor_tensor(out=t1[:], in0=xr, in1=sk,
                                            op=mybir.AluOpType.mult)
                    nc.vector.tensor_tensor(out=t2[:], in0=xi, in1=ck,
                                            op=mybir.AluOpType.mult)
                    nc.vector.tensor_tensor(out=t1[:], in0=t1[:], in1=hprev_i,
                                            op=mybir.AluOpType.add)
                    nc.vector.tensor_tensor(out=Hs[:, 1, k - 1, :], in0=t1[:],
                                            in1=t2[:], op=mybir.AluOpType.add)

                # ---- u = x_0 @ W_top (fp16)
                u16 = cold.tile([P, 4, NC], f16, tag="u16")
                for mb in range(4):
                    pu = ps_big.tile([P, NC], f32, tag="big")
                    for kb in range(2):
                        nc.tensor.matmul(
                            pu[:], wt16[:, kb, mb * P : (mb + 1) * P],
                            x0c[:, kb, :], start=(kb == 0), stop=(kb == 1))
                    nc.scalar.copy(u16[:, mb, :], pu[:])

                # ---- attention P_j + tanh + ic-dot + per-j exp
                e_all = cold.tile([1, L, NC], f32, tag="e_all")
                for j in range(1, 8):
                    psj = ps_s.tile([P, NC], f32, tag="s")
                    a4 = cold.tile([P, 4, NC], f16, tag="a4")
                    for mb in range(4):
                        pP = ps_big.tile([P, NC], f32, tag="big")
                        for kb in range(2):
                            nc.tensor.matmul(
                                pP[:], wb16[:, kb, mb * P : (mb + 1) * P],
                                Hs[:, kb, j - 1, :],
                                start=(kb == 0), stop=False)
                        nc.tensor.matmul(
                            pP[:], I7[:, j - 1, :], u16[:, mb, :],
                            start=False, stop=True)
                        nc.scalar.activation(
                            a4[:, mb, :], pP[:], mybir.ActivationFunctionType.Tanh,
                            bias=ab_col[:, mb : mb + 1], scale=float(1.0 / (j + 1)))
                    for mb in range(4):
                        nc.tensor.matmul(
                            psj[:], icp[:, mb, :], a4[:, mb, :],
                            start=(mb == 0), stop=(mb == 3))
                    nc.scalar.activation(e_all[:, j - 1, :], psj[0:1, :],
                                         mybir.ActivationFunctionType.Exp,
                                         bias=zcol[0:1, 0:1], scale=1.0)

                # ---- softmax over j (per-row) + weighted H sum + hid accum
                esum = cold.tile([1, NC], f32, tag="esum")
                nc.vector.tensor_tensor(out=esum[:], in0=e_all[:, 0, :],
                                        in1=e_all[:, 1, :], op=mybir.AluOpType.add)
                for j in range(3, 8):
                    nc.vector.tensor_tensor(out=esum[:], in0=esum[:],
                                            in1=e_all[:, j - 1, :],
                                            op=mybir.AluOpType.add)
                rec = cold.tile([1, NC], f32, tag="rec")
                nc.vector.reciprocal(rec[:], esum[:])
                wdd_all = cold.tile([1, L, NC], f16, tag="wdd_all")
                for j in range(1, 8):
                    wj = cold.tile([1, NC], f32, tag="wj")
                    nc.vector.tensor_tensor(out=wj[:], in0=e_all[:, j - 1, :],
                                            in1=rec[:], op=mybir.AluOpType.mult)
                    nc.vector.tensor_scalar_mul(wdd_all[:, j - 1, :], wj[:],
                                                float(1.0 / (j + 1)))

                mh = cold.tile([P, 2, NC], f16, tag="mh")
                for j in range(1, 8):
                    wbc = sp.tile([P, NC], f16, tag="wbc")
                    nc.gpsimd.partition_broadcast(wbc[:], wdd_all[:, j - 1, :])
                    for fb in range(2):
                        if j == 1:
                            nc.vector.tensor_tensor(
                                out=mh[:, fb, :], in0=Hs[:, fb, 0, :],
                                in1=wbc[:], op=mybir.AluOpType.mult)
                        else:
                            tmp = sp.tile([P, NC], f16, tag="tmp")
                            nc.vector.tensor_tensor(
                                out=tmp[:], in0=Hs[:, fb, j - 1, :],
                                in1=wbc[:], op=mybir.AluOpType.mult)
                            nc.vector.tensor_tensor(
                                out=mh[:, fb, :], in0=mh[:, fb, :], in1=tmp[:],
                                op=mybir.AluOpType.add)
                m = c // 2
                for ah in range(2):
                    cols = slice(ah * BLOC, (ah + 1) * BLOC)
                    first = (c % 2 == 0) and (ah == 0)
                    for g2 in range(2):  # 0: top (x0), 1: bottom (mh)
                        srct = x0c[:, :, cols] if g2 == 0 else mh[:, :, cols]
                        rel = cold.tile([P, 2, BLOC], f32r, tag="rel")
                        nc.vector.tensor_scalar(
                            out=rel[:], in0=srct, scalar1=0.0, scalar2=None,
                            op0=mybir.AluOpType.max)
                        dst = hid[:, m, 2 * g2 : 2 * g2 + 2, :]
                        if first:
                            nc.vector.tensor_copy(dst, rel[:])
                        else:
                            nc.vector.tensor_tensor(
                                out=dst, in0=dst, in1=rel[:],
                                op=mybir.AluOpType.add)

            # ---- inter attention: t_m = sum_b tanh(hid_m @ inter_W + ib)
            tpart = pp.tile([P, 4, M], f32)
            scratch = p2.tile([P, BLOC], f32, tag="scr")
            for m in range(M):
                for mb in range(4):
                    pt = ps_big.tile([P, BLOC], f32, tag="big")
                    for kb in range(4):
                        nc.tensor.matmul(
                            pt[:], iw_r[:, kb, mb * P : (mb + 1) * P],
                            hid[:, m, kb, :], start=(kb == 0), stop=(kb == 3))
                    nc.scalar.activation(
                        scratch[:], pt[:], mybir.ActivationFunctionType.Tanh,
                        bias=ib_col[:, mb : mb + 1], scale=1.0,
                        accum_out=tpart[:, mb, m : m + 1])

            # ---- AllReduce t over cores
            t_in = dp.tile([P, 4 * M], f32)
            t_out = dp.tile([P, 4 * M], f32, addr_space="Shared")
            nc.sync.dma_start(t_in[:], tpart[:].rearrange("p a b -> p (a b)"))
            nc.gpsimd.collective_compute(
                "AllReduce", mybir.AluOpType.add,
                replica_groups=[list(range(NCORES))],
                ins=[t_in[:]], outs=[t_out[:]])
            tsum = p2.tile([P, 4, M], f32, tag="tsum")
            nc.sync.dma_start(tsum[:], t_out[:].rearrange("p (a b) -> p a b", a=4))
            tsum_r = p2.tile([P, 4, M], f32, tag="tsr")
            nc.vector.tensor_scalar_mul(tsum_r[:], tsum[:], float(1.0 / B))

            # ---- scores
            psc = ps_s.tile([1, M], f32, tag="sc")
            for kb in range(4):
                nc.tensor.matmul(psc[:], ic2_r[:, kb : kb + 1], tsum_r[:, kb, :],
                                 start=(kb == 0), stop=(kb == 3))
            sc_sb = p2.tile([1, M], f32, tag="scsb")
            nc.vector.tensor_copy(sc_sb[:], psc[:])
            scb = p2.tile([P, M], f32, tag="scb")
            nc.gpsimd.partition_broadcast(scb[:], sc_sb[:])

            # ---- out_pre = sum_m hid_m * scores_m ; head matmul
            op_r = p2.tile([P, 4, BLOC], f32r, tag="opr")
            tmp2 = p2.tile([P, 4, BLOC], f32r, tag="tmp2")
            for m in range(M):
                dst = op_r if m == 0 else tmp2
                nc.vector.tensor_scalar(
                    out=dst[:], in0=hid[:, m, :, :], scalar1=scb[:, m : m + 1],
                    scalar2=None, op0=mybir.AluOpType.mult)
                if m > 0:
                    nc.vector.tensor_tensor(out=op_r[:], in0=op_r[:], in1=tmp2[:],
                                            op=mybir.AluOpType.add)
            outsb = p2.tile([P, 2, BLOC], f32, tag="outsb")
            for mb in range(2):
                po = ps_big.tile([P, BLOC], f32, tag="big")
                for kb in range(4):
                    nc.tensor.matmul(
                        po[:], ow_r[:, kb, mb * P : (mb + 1) * P],
                        op_r[:, kb, :], start=(kb == 0), stop=(kb == 3))
                nc.scalar.activation(
                    outsb[:, mb, :], po[:], mybir.ActivationFunctionType.Identity,
                    bias=ob_col[:, mb : mb + 1], scale=1.0)
            nc.sync.dma_start(OUT[:].rearrange("b p n -> p b n"), outsb[:])

    nc.compile()
    return nc


# ---------------- host side ----------------

def _host_prep(inputs):
    tokens = np.asarray(inputs["tokens"]).astype(np.int32)       # [3,4,2048,8]
    etok = np.asarray(inputs["edge_tokens"]).astype(np.int32)    # [3,4,2048,7]
    consts = {
        "identc": np.eye(P, dtype=np.float32),
        "sel112": np.repeat(np.eye(L, dtype=np.float16), 16, axis=1),
        "iota112": np.tile(np.arange(16, dtype=np.float32), L)[:, None],
        "jdiv7": (1.0 / np.arange(2, 9, dtype=np.float32))[:, None],
        "ones7": np.ones((L, 1), np.float32),
    }
    rep = {k: np.ascontiguousarray(np.asarray(inputs[k]).astype(np.float32))
           for k in ["edge_emb", "scale_W", "scale_b", "attn_W",
                      "attn_b", "inter_W", "inter_b", "out_W", "out_b"]}
    rep["node_emb"] = np.ascontiguousarray(
        np.asarray(inputs["node_emb"]).astype(np.float16))
    rep["intra_c"] = np.asarray(inputs["intra_context"]).astype(np.float32)
    rep["ee_rep"] = np.ascontiguousarray(
        np.tile(np.asarray(inputs["edge_emb"]).astype(np.float32), (L, 1)))
    rep["inter_c"] = np.asarray(inputs["inter_context"]).astype(np.float32)

    in_maps = []
    for core in range(NCORES):
        bs = slice(core * BLOC, (core + 1) * BLOC)
        tok_r = tokens[:, :, bs, :].reshape(R, L + 1)            # r=(m,a,b)
        et_r = etok[:, :, bs, :].reshape(R, L)
        # gidx[p, c*32 + k*4 + q] = tok_r[c*512 + q*128 + p, k]
        t4 = tok_r.reshape(NCH, 4, P, L + 1)                     # [c, q, p, k]
        gidx = np.ascontiguousarray(
            t4.transpose(2, 0, 3, 1).reshape(P, NCH * (L + 1) * 4))
        etokT = np.ascontiguousarray(et_r.T)                     # [7, R]
        m = {"gidx": gidx, "etokT": etokT}
        m.update(consts)
        m.update(rep)
        in_maps.append(m)
    return in_maps


def _get_runner():
    if "runner" not in _CACHE:
        nc = build_nc()
        from concourse import bass2jax  # noqa
        import jax
        import jax.numpy as jnp
        from jax.experimental.shard_map import shard_map
        from jax.sharding import Mesh, PartitionSpec
        from concourse.bass2jax import (_bass_exec_p, install_neuronx_cc_hook,
                                        partition_id_tensor)
        install_neuronx_cc_hook()
        partition_name = (nc.partition_id_tensor.name
                          if nc.partition_id_tensor else None)
        in_names, out_names, out_avals = [], [], []
        for alloc in nc.m.functions[0].allocations:
            if not isinstance(alloc, mybir.MemoryLocationSet):
                continue
            name = alloc.memorylocations[0].name
            if alloc.kind == "ExternalInput":
                if name != partition_name:
                    in_names.append(name)
            elif alloc.kind == "ExternalOutput":
                shape = tuple(alloc.tensor_shape)
                npdt = mybir.dt.np(alloc.dtype)
                out_names.append(name)
                out_avals.append(jax.core.ShapedArray(shape, npdt))
        n_params, n_outs = len(in_names), len(out_avals)
        all_in = in_names + out_names + ([partition_name] if partition_name else [])

        def _body(*args):
            operands = list(args)
            if partition_name is not None:
                operands.append(partition_id_tensor())
            return tuple(_bass_exec_p.bind(
                *operands, out_avals=tuple(out_avals), in_names=tuple(all_in),
                out_names=tuple(out_names), lowering_input_output_aliases=(),
                sim_require_finite=True, sim_require_nnan=True, nc=nc))

        devices = jax.devices()[:NCORES]
        mesh = Mesh(np.asarray(devices), ("core",))
        fn = jax.jit(
            shard_map(_body, mesh=mesh,
                      in_specs=(PartitionSpec("core"),) * (n_params + n_outs),
                      out_specs=(PartitionSpec("core"),) * n_outs,
                      check_rep=False),
            donate_argnums=tuple(range(n_params, n_params + n_outs)),
            keep_unused=True)
        _CACHE["runner"] = (fn, in_names, out_names, out_avals)
    return _CACHE["runner"]


def run_device(in_maps):
    fn, in_names, out_names, out_avals = _get_runner()
    concat = [np.concatenate([np.asarray(in_maps[c][n]) for c in range(NCORES)],
                             axis=0) for n in in_names]
    zeros = [np.zeros((NCORES * a.shape[0], *a.shape[1:]), a.dtype)
             for a in out_avals]
    outs = fn(*concat, *zeros)
    outs = [np.asarray(o) for o in outs]
    return [
        {name: outs[i].reshape(NCORES, *out_avals[i].shape)[c]
         for i, name in enumerate(out_names)}
        for c in range(NCORES)
    ]


def kernel(**inputs):
    in_maps = _host_prep(inputs)
    res = run_device(in_maps)
    full = np.empty((B, H), np.float32)
    for core in range(NCORES):
        o = res[core]["OUT"]                      # [2, 128, 256] = [mb, p, b]
        full[core * BLOC : (core + 1) * BLOC, :] = (
            o.transpose(2, 0, 1).reshape(BLOC, H))
    return full

